# revision 82
# baseline (speedup 1.0000x reference)
"""Trainium2 Bass kernel for the CSMAdapter module.

Contract: kernel(**inputs) takes the FULL unsharded inputs (as produced by
the reference setup_inputs()) and returns the FULL output [4, 100, 1024].

Strategy
--------
All weight-only computation is folded on the host (it is data-independent):
    w_proj   = W_in @ Wd.T + bd
    w_prime  = P.T @ w_proj @ P
    masked_w = w_prime * sigmoid(spectral_mask)
    A        = P @ masked_w.T @ P.T          # fused = x @ A
    W_big    = W_in.T @ A                    # fused = llama @ W_big + b_in @ A
The final LayerNorm + mel projection algebra is folded into the mel GEMM:
    mel[m,t] = rstd[t]*((Wg @ h2)[m,t] - mu[t]*c1[m]) + c2[m]
with Wg = Wmel * ln_g, c1 = Wmel @ ln_g, c2 = Wmel @ ln_b + bmel.
The -mu*c1 term is accumulated into the mel PSUM as a rank-1 matmul
(c1 (x) -mu/D*Dsum) so the tail is: rstd broadcast -> one multiply ->
one per-partition bias add -> output DMA.

Device (SPMD over 8 cores, data-parallel over the 4096 tokens, 512 each +
2-token conv halos).  All heavy matmuls run in bf16; PSUM stays fp32.

W_big chunks for d0/d1, d2/d3 and d6/d7 are stored fp8 (e4m3) with
per-output-channel scales folded into the fu dequant (the PE multiplies
fp8 stationary x bf16 moving at full rate), halving most of the weight
DMA; total quantization error is ~1.5e-2 of scale vs the 2e-2 gate.

DMA: the two HW DGE queues (sync + scalar) stream wbc0 k-sliced and the x
sub-chunks interleaved so the first GEMMs start right as the warmup ends;
then the rest of x, wbc1, wbc2.  The gpsimd software-DGE queue carries
wbc3 (d6/d7 fill the x-paced head), consts (halo columns land directly in
the fuall/gall slabs), then conv/mel weights.  All queues share the 16
physical DMA engines (~210 GB/s aggregate), so ordering (not lane count)
is what matters.  The LN tail computes rstd = sqrt(recip(var+eps)) with
recip on the vector engine back-to-back after var, and the Sqrt act table
is pre-warmed by a dummy activation whose input aliases the last Gelu's
output (so the dependency scheduler cannot hoist it before the Gelu-table
load, which would re-evict Sqrt).
"""

import sys

import numpy as np


def _ensure_concourse():
    try:
        import concourse  # noqa: F401
    except ImportError:  # pragma: no cover
        for p in ("/opt/trn_rl_repo", "/root/.axon_site/_ro/trn_rl_repo"):
            if p not in sys.path:
                sys.path.insert(0, p)


# ---- static shapes ----
B, T, L, D = 4, 1024, 3072, 1024
NCORES = 8
TOK = 512            # owned tokens per core
EXT = TOK + 4        # fused ext window: tokens -2 .. TOK+2
G1E = TOK + 2        # conv1 ext output: tokens -1 .. TOK+1
KT = L // 128        # 24
KH = KT // 2         # 12
DT = D // 128        # 8
NMEL = 100
GS = 64              # group size (1024 / 16 groups)
GROUPS_ = 16

# cb (per-partition constants) column layout
CB_B1 = 0            # conv1 bias (gelu bias), 8 cols
CB_B2S = 8           # conv2 bias / 32 (Square path), 8 cols
CB_B2 = 16           # conv2 bias, 8 cols
CB_EPS = 24          # LN eps
CB_BBIG = 25         # fused GEMM bias b_big, 8 cols
CB_C2 = 33           # mel output bias c2 (partitions 0..99)
CB_WSC = 34          # fp8 weight dequant scale per d-tile, 8 cols
CB_LEN = 42

# d-tiles whose W_big columns are stored fp8 (e4m3, per-column scaled).
# Three of four chunks keeps the added quantization error at ~1.5e-2 of
# scale (vs the 2e-2 gate, measured 1.24e-2 with two chunks); these are the
# front/mid-of-stream tiles so the DMA-bound kernel head shrinks the most.
FP8_CHUNKS = (0, 1, 3)       # wbig chunks for d0/d1, d2/d3, d6/d7

LN_EPS = 1e-5
N_WU = 0             # PE warmup matmuls (pstate ramp during input DMA)

_PROGRAM = None          # cached program
LAST_RESULTS = None      # BassKernelResults of the most recent run (for test.py)


def _build_program():
    _ensure_concourse()
    from concourse import bacc, tile
    import concourse.mybir as mybir

    f32 = mybir.dt.float32
    f32r = mybir.dt.float32r
    bf16 = mybir.dt.bfloat16
    fp8 = mybir.dt.float8e4
    AF = mybir.ActivationFunctionType
    MUL = mybir.AluOpType.mult
    SUB = mybir.AluOpType.subtract

    nc = bacc.Bacc("TRN2", debug=False, target_bir_lowering=False)

    # DRAM layouts are partition-major so every DMA is contiguous.
    # wbig chunk layout is k-major: [chunk, kp, ktile, dpair, dcol]
    xt_d = nc.dram_tensor("xt", [2, 128, KH, EXT], bf16, kind="ExternalInput")
    xt8_d = nc.dram_tensor("xt8", [128, 4, EXT], fp8, kind="ExternalInput")
    wbig8_d = nc.dram_tensor("wbig8", [3, 128, KT, 2, 128], fp8,
                             kind="ExternalInput")
    wbig_d = nc.dram_tensor("wbig", [1, 128, KT, 2, 128], bf16,
                            kind="ExternalInput")
    cw1_d = nc.dram_tensor("cw1", [128, DT, 3, 128], bf16, kind="ExternalInput")
    cw2_d = nc.dram_tensor("cw2", [128, DT, 3, 128], bf16, kind="ExternalInput")
    wgt_d = nc.dram_tensor("wgt", [128, DT, NMEL + 1], bf16,
                           kind="ExternalInput")
    cb_d = nc.dram_tensor("cb", [128, CB_LEN], f32, kind="ExternalInput")
    smb_d = nc.dram_tensor("smb", [1, 2 * NMEL], bf16, kind="ExternalInput")
    onec_d = nc.dram_tensor("onec", [128, 1], bf16, kind="ExternalInput")
    # host-computed halo columns: per d-tile, 4 fused halo cols + 2 g halo cols
    halo_d = nc.dram_tensor("halo", [128, DT, 6], bf16, kind="ExternalInput")
    mel_d = nc.dram_tensor("mel", [NMEL, TOK], bf16, kind="ExternalOutput")

    with tile.TileContext(nc) as tc:
        with (
            tc.tile_pool(name="consts", bufs=1) as consts,
            tc.tile_pool(name="wpool", bufs=1) as wpool,
            tc.tile_pool(name="acts", bufs=1) as acts,
            tc.tile_pool(name="stats", bufs=1) as stats,
            tc.tile_pool(name="ps_mm", bufs=6, space="PSUM") as ps_mm,
            tc.tile_pool(name="ps_sq", bufs=1, space="PSUM") as ps_sqp,
            tc.tile_pool(name="ps_mel", bufs=1, space="PSUM") as ps_melp,
        ):
            fuall = acts.tile([128, DT, EXT], bf16, name="fuall")
            gall = acts.tile([128, DT, G1E], bf16, name="gall")

            # ---- input DMAs (see module docstring) ----
            def load_split(t, src_ap):
                nc.sync.dma_start(out=t[0:64], in_=src_ap[0:64])
                nc.scalar.dma_start(out=t[64:128], in_=src_ap[64:128])

            wbc = [
                wpool.tile([128, KT, 2, 128],
                           fp8 if c in FP8_CHUNKS else bf16,
                           name=f"wbc{c}", tag=f"wbc{c}")
                for c in range(4)
            ]
            xg = [
                consts.tile([128, KH, EXT], bf16, name=f"xg{j}", tag=f"xg{j}")
                for j in range(2)
            ]
            cw1_sb = consts.tile([128, DT, 3, 128], bf16, name="cw1_sb")
            cw2_sb = consts.tile([128, DT, 3, 128], bf16, name="cw2_sb")
            wgt_sb = consts.tile([128, DT, NMEL + 1], bf16, name="wgt_sb")

            # HW queues: wbc0 (fp8) k-sliced against the first x sub-chunks
            # so the d0/d1 GEMMs can start as soon as the warmup ends, then
            # the rest of x, then wbc1 (fp8) and wbc2 (bf16).
            xg8 = consts.tile([128, 4, EXT], fp8, name="xg8")
            load_split(xg8, xt8_d[:])
            load_split(wbc[0][:, 0:8], wbig8_d[0][:, 0:8])
            # bf16 copy of the fp8 k0-3 chunk for the later (bf16-lhsT)
            # d-tiles; runs on the idle vector engine right after arrival.
            nc.vector.tensor_copy(xg[0][:, 0:4, :], xg8)
            load_split(xg[0][:, 4:8, :], xt_d[0][:, 4:8, :])
            load_split(wbc[0][:, 8:16], wbig8_d[0][:, 8:16])
            load_split(xg[0][:, 8:12, :], xt_d[0][:, 8:12, :])
            load_split(xg[1][:, 0:4, :], xt_d[1][:, 0:4, :])
            load_split(wbc[0][:, 16:24], wbig8_d[0][:, 16:24])
            load_split(xg[1][:, 4:8, :], xt_d[1][:, 4:8, :])
            load_split(xg[1][:, 8:12, :], xt_d[1][:, 8:12, :])
            load_split(wbc[1], wbig8_d[1])
            load_split(wbc[2], wbig_d[0])
            # gpsimd lane: wbc3 (fp8) first so d6/d7 can fill the x-paced
            # head, then the consts (needed from fu0 ~28us), then conv/mel
            # weights.
            nc.gpsimd.dma_start(out=wbc[3], in_=wbig8_d[2])
            cb_sb = consts.tile([128, CB_LEN], f32, name="cb_sb")
            nc.gpsimd.dma_start(out=cb_sb, in_=cb_d[:])
            ones_col = consts.tile([128, 1], bf16, name="ones_col")
            nc.gpsimd.dma_start(out=ones_col, in_=onec_d[:])
            smb_sb = consts.tile([1, 2 * NMEL], bf16, name="smb_sb")
            nc.gpsimd.dma_start(out=smb_sb, in_=smb_d[:])
            # host halo columns DMAed straight into the fused / conv1 slabs
            # (replaces 32 small vector copies in the conv phase).
            nc.gpsimd.dma_start(out=fuall[:, :, 0:2], in_=halo_d[:, :, 0:2])
            nc.gpsimd.dma_start(out=fuall[:, :, EXT - 2 : EXT],
                                in_=halo_d[:, :, 2:4])
            nc.gpsimd.dma_start(out=gall[:, :, 0:1], in_=halo_d[:, :, 4:5])
            nc.gpsimd.dma_start(out=gall[:, :, G1E - 1 : G1E],
                                in_=halo_d[:, :, 5:6])
            nc.gpsimd.dma_start(out=cw1_sb, in_=cw1_d[:])
            nc.gpsimd.dma_start(out=cw2_sb, in_=cw2_d[:])
            nc.gpsimd.dma_start(out=wgt_sb, in_=wgt_d[:])

            def xk(k):
                return xg[k // KH][:, k % KH, :]

            h2 = [None] * DT
            h2sq = [None] * DT
            ps_sq_ref = [None]
            ps_m_ref = [None]
            psA = {}

            def gemm_chunk(d, ka, kb):
                if d not in psA:
                    psA[d] = ps_mm.tile([128, TOK], f32, name=f"psA{d}",
                                        tag="mm")
                for k in range(ka, kb):
                    if k < 4 and (d // 2) in (0, 3):
                        # fp8 lhsT tiles consume the fp8 first-chunk rhs
                        # directly (no wait on the bf16 cast)
                        rhs = xg8[:, k, 2 : 2 + TOK]
                    else:
                        rhs = xk(k)[:, 2 : 2 + TOK]
                    nc.tensor.matmul(
                        psA[d],
                        lhsT=wbc[d // 2][:, k, d % 2, :],
                        rhs=rhs,
                        start=(k == 0), stop=(k == KT - 1),
                    )

            def fu_copy(d, eng=None):
                # fu = psA * wscale + b_big; wscale is the fp8 per-channel
                # dequant scale (1.0 for the bf16 d-tiles).  Late tiles run
                # on the vector engine to debottleneck the scalar engine in
                # the conv phase.
                if eng is None:
                    nc.scalar.activation(
                        out=fuall[:, d, 2 : 2 + TOK], in_=psA[d],
                        func=AF.Identity,
                        bias=cb_sb[:, CB_BBIG + d : CB_BBIG + d + 1],
                        scale=cb_sb[:, CB_WSC + d : CB_WSC + d + 1],
                    )
                else:
                    eng.tensor_scalar(
                        fuall[:, d, 2 : 2 + TOK], psA[d],
                        cb_sb[:, CB_WSC + d : CB_WSC + d + 1],
                        cb_sb[:, CB_BBIG + d : CB_BBIG + d + 1],
                        MUL, mybir.AluOpType.add,
                    )

            def conv1(d):
                # device computes g_ext cols [1, 513); cols 0 and 513 from host
                ps = ps_mm.tile([128, TOK], f32, name=f"psB{d}", tag="mm")
                for tap in range(3):
                    nc.tensor.matmul(
                        ps, lhsT=cw1_sb[:, d, tap, :],
                        rhs=fuall[:, d, 1 + tap : 1 + tap + TOK],
                        start=(tap == 0), stop=(tap == 2),
                    )
                nc.scalar.activation(
                    out=gall[:, d, 1 : 1 + TOK], in_=ps, func=AF.Gelu,
                    bias=cb_sb[:, CB_B1 + d : CB_B1 + d + 1], scale=1.0,
                )

            def conv2(d):
                h2d = acts.tile([128, TOK], bf16, name=f"h2{d}", tag=f"h2{d}")
                h2sqd = acts.tile([128, TOK], bf16, name=f"h2sq{d}", tag="h2sq",
                                  bufs=2)
                h2[d] = h2d
                h2sq[d] = h2sqd
                ps = ps_mm.tile([128, TOK], f32, name=f"psC{d}", tag="mm")
                for tap in range(3):
                    nc.tensor.matmul(
                        ps, lhsT=cw2_sb[:, d, tap, :],
                        rhs=gall[:, d, tap : tap + TOK],
                        start=(tap == 0), stop=(tap == 2),
                    )
                nc.vector.tensor_scalar_add(
                    h2d, ps, cb_sb[:, CB_B2 + d : CB_B2 + d + 1])
                # h2sq = ((ps + b2)/32)^2 = h2^2 / 1024 -> ps_sq row = E[h2^2]
                nc.scalar.activation(
                    out=h2sqd, in_=ps, func=AF.Square,
                    bias=cb_sb[:, CB_B2S + d : CB_B2S + d + 1], scale=0.03125,
                )

            stat_n = [0]

            def statmm(d):
                # token-wise sums of h2 and h2^2; the mel matmuls are
                # deferred past the stats so the LN chain starts earlier.
                first = stat_n[0] == 0
                stat_n[0] += 1
                last = stat_n[0] == DT
                if first:
                    ps_sq_ref[0] = ps_sqp.tile([33, TOK], f32, name="ps_sq")
                nc.tensor.matmul(ps_sq_ref[0][0:1, :], lhsT=ones_col,
                                 rhs=h2sq[d][:], start=first, stop=last)
                nc.tensor.matmul(ps_sq_ref[0][32:33, :], lhsT=ones_col,
                                 rhs=h2[d][:], start=first, stop=last)

            # ---- emission in expected DMA-arrival order ----
            # d0/d1 interleave against the early x sub-chunks; d6/d7 (whose
            # weights arrive early on the gpsimd lane) join to fill the gaps
            # between x sub-chunk arrivals; d2..d5 follow weight arrival.
            gemm_chunk(0, 0, 4)
            gemm_chunk(1, 0, 4)
            gemm_chunk(0, 4, 8)
            gemm_chunk(1, 4, 8)
            gemm_chunk(6, 0, 4)
            gemm_chunk(7, 0, 4)
            gemm_chunk(6, 4, 8)
            gemm_chunk(7, 4, 8)
            gemm_chunk(0, 8, 12)
            gemm_chunk(1, 8, 12)
            gemm_chunk(6, 8, 12)
            gemm_chunk(7, 8, 12)
            for c in (3, 4, 5):
                for d in (0, 1, 6, 7):
                    gemm_chunk(d, 4 * c, 4 * c + 4)
            fu_copy(0)
            fu_copy(1)
            fu_copy(6)
            fu_copy(7)
            conv1(0)
            gemm_chunk(2, 0, KT)
            fu_copy(2, eng=nc.vector)
            conv1(1)
            gemm_chunk(3, 0, KT)
            fu_copy(3, eng=nc.vector)
            conv1(6)
            conv2(0)
            gemm_chunk(4, 0, KT)
            fu_copy(4, eng=nc.vector)
            conv1(7)
            conv2(1)
            statmm(0)
            gemm_chunk(5, 0, KT)
            fu_copy(5, eng=nc.vector)
            conv1(2)
            conv2(6)
            statmm(1)
            conv1(3)
            conv2(7)
            statmm(6)
            conv1(4)
            conv2(2)
            statmm(7)
            conv1(5)
            # pre-warm the Sqrt act table right after the last Gelu - the
            # input is gall's d5 slice so the dependency-driven scheduler
            # cannot hoist this above conv1(5)'s Gelu (whose own table load
            # would re-evict Sqrt).  Square is present in every table set so
            # the remaining h2sq squares are unaffected, and the LN-tail
            # Sqrt then needs no load on the critical path.
            pre = stats.tile([1, 8], f32, name="pre", tag="pre", bufs=2)
            nc.scalar.activation(pre, gall[0:1, 5, 0:8], AF.Sqrt,
                                 bias=cb_sb[0:1, CB_EPS : CB_EPS + 1],
                                 scale=1.0)
            conv2(3)
            statmm(2)
            conv2(4)
            statmm(3)
            conv2(5)
            statmm(4)
            statmm(5)

            # ---- deferred mel matmuls (overlap the LN stats chain) ----
            ps_m = ps_melp.tile([NMEL, TOK], f32, name="ps_m")
            ps_m_ref[0] = ps_m
            for i, d in enumerate(range(DT)):
                nc.tensor.matmul(ps_m, lhsT=wgt_sb[:, d, 0:NMEL],
                                 rhs=h2[d][:], start=(i == 0), stop=False)

            # ---- LN stats on [1, TOK] ----
            ps_sq = ps_sq_ref[0][0:1, :]     # E[h2^2] per token
            mu_row = ps_sq_ref[0][32:33, :]  # sum h2; x(1/D) folded into ops
            negmu = stats.tile([1, TOK], bf16, name="negmu")
            msq = stats.tile([1, TOK], f32, name="msq")
            var = stats.tile([1, TOK], f32, name="var", tag="sv", bufs=2)
            rvar = stats.tile([1, TOK], f32, name="rvar", tag="sv", bufs=2)
            rstd = stats.tile([1, TOK], bf16, name="rstd")
            # column-halved software pipeline; var and recip are back-to-back
            # on the vector engine, and the final Sqrt writes bf16 directly:
            #   rstd = sqrt(1/(E[h^2] + eps - mu^2))
            HT = TOK // 2
            Q3 = HT + TOK // 4
            for h in range(2):
                s = slice(h * HT, (h + 1) * HT)
                nc.scalar.activation(msq[0:1, s], mu_row[0:1, s], AF.Square,
                                     scale=1.0 / D)
                nc.vector.scalar_tensor_tensor(
                    var[0:1, s], in0=ps_sq[0:1, s], scalar=LN_EPS,
                    in1=msq[0:1, s], op0=mybir.AluOpType.add, op1=SUB,
                )
                nc.vector.reciprocal_approx_fast(rvar[0:1, s], var[0:1, s])
                nc.scalar.activation(rstd[0:1, s], rvar[0:1, s], AF.Sqrt,
                                     bias=0.0, scale=1.0)

            # ---- rstd broadcast + output assembly (2-half pipeline) ----
            # The ps_s broadcasts are emitted BEFORE the rank-1 correction:
            # the in-order PE would otherwise block on rank1 (whose negmu
            # input is last on the vector queue) while rstd h0 sat ready.
            ps_s = ps_mm.tile([NMEL, TOK], f32, name="ps_s", tag="mm")
            s_sb = stats.tile([NMEL, TOK], bf16, name="s_sb")
            out_sb = stats.tile([NMEL, TOK], bf16, name="out_sb")
            c2col = cb_sb[0:NMEL, CB_C2 : CB_C2 + 1]
            for h in range(2):
                s = slice(h * HT, (h + 1) * HT)
                nc.tensor.matmul(
                    ps_s[:, s], lhsT=smb_sb[0:1, NMEL : 2 * NMEL],
                    rhs=rstd[0:1, s], start=True, stop=True,
                )
            nc.vector.tensor_scalar_mul(negmu, mu_row, -1.0 / D)
            nc.tensor.matmul(
                ps_m[0:NMEL, :], lhsT=smb_sb[0:1, 0:NMEL],
                rhs=negmu, start=False, stop=True,
            )
            for h in range(2):
                s = slice(h * HT, (h + 1) * HT)
                nc.scalar.copy(s_sb[:, s], ps_s[:, s])
                nc.vector.tensor_mul(out_sb[:, s], ps_m[0:NMEL, s],
                                     s_sb[:, s])
                nc.scalar.add(out=out_sb[:, s], in_=out_sb[:, s], add=c2col)
            nc.sync.dma_start(out=mel_d[:, 0:HT], in_=out_sb[:, 0:HT])
            nc.scalar.dma_start(out=mel_d[:, HT:Q3], in_=out_sb[:, HT:Q3])
            nc.gpsimd.dma_start(out=mel_d[:, Q3:TOK], in_=out_sb[:, Q3:TOK])

    nc.compile()
    return nc


def _sigmoid64(x):
    return 1.0 / (1.0 + np.exp(-x.astype(np.float64)))


def _bf16(a):
    import ml_dtypes

    return np.ascontiguousarray(np.asarray(a, dtype=np.float32)).astype(
        ml_dtypes.bfloat16
    )


def host_prep(inputs):
    """Fold all data-independent computation; build per-core device inputs.

    Returns (shared, per_core) where shared is a dict of replicated arrays
    and per_core is a list of 8 dicts with the core-specific arrays.
    """
    f32 = np.float32
    W_in = np.asarray(inputs["W_in"], dtype=np.float64)
    Wd = np.asarray(inputs["Wd"], dtype=np.float64)
    bd = np.asarray(inputs["bd"], dtype=np.float64)
    P = np.asarray(inputs["P"], dtype=np.float64)
    smask = np.asarray(inputs["spectral_mask"], dtype=np.float64)
    b_in = np.asarray(inputs["b_in"], dtype=np.float64)

    w_proj = W_in @ Wd.T + bd[None, :]
    w_prime = P.T @ w_proj @ P
    masked_w = w_prime * _sigmoid64(smask)
    A = P @ masked_w.T @ P.T
    W_big64 = W_in.T @ A                                       # [L, D] f64
    b_big64 = b_in @ A                                         # [D] f64
    W_big = np.ascontiguousarray(W_big64, dtype=f32)

    # [chunk of 2 d-tiles, kp, ktile, d%2, dc] (partition-major, k-major)
    import ml_dtypes

    wchunks = W_big.reshape(KT, 128, 4, 2, 128).transpose(2, 1, 0, 3, 4)
    # fp8 chunks: per-output-column scales (folded into the fu dequant)
    wscale = np.ones((DT, 128), dtype=f32)
    w8_list = []
    w16_list = []
    for c in range(4):
        if c in FP8_CHUNKS:
            cols64 = W_big64[:, c * 256 : (c + 1) * 256]     # [L, 256]
            amax = np.abs(cols64).max(axis=0)
            s = (amax / 224.0).astype(f32)
            s[s == 0] = 1.0
            wscale[2 * c] = s[0:128]
            wscale[2 * c + 1] = s[128:256]
            sc = s.reshape(2, 128)[None, None, :, :]          # [1,1,2,128]
            w8_list.append(
                (wchunks[c] / sc).astype(np.float32).astype(
                    ml_dtypes.float8_e4m3
                )
            )
        else:
            w16_list.append(_bf16(wchunks[c]))
    wbig8_t = np.stack(w8_list, axis=0)
    wbig_t = np.stack(w16_list, axis=0)

    def blockdiag(w):
        w = np.asarray(w, dtype=f32)  # [C, GS, 3]
        out = np.zeros((DT, 3, 128, 128), dtype=f32)
        for d in range(DT):
            for co in range(128):
                c = d * 128 + co
                blk = co // GS
                # out[d, tap, blk*GS + i, co] = w[c, i, tap]
                out[d, :, blk * GS : (blk + 1) * GS, co] = w[c].T
        return out

    cw1_t = _bf16(blockdiag(inputs["conv1_w"]).transpose(2, 0, 1, 3))
    cw2_t = _bf16(blockdiag(inputs["conv2_w"]).transpose(2, 0, 1, 3))

    Wmel = np.asarray(inputs["Wmel"], dtype=np.float64)
    ln_g = np.asarray(inputs["ln_g"], dtype=np.float64)
    ln_b = np.asarray(inputs["ln_b"], dtype=np.float64)
    bmel = np.asarray(inputs["bmel"], dtype=np.float64)
    Wg = (Wmel * ln_g[None, :]).astype(f32)                    # [NMEL, D]
    wgt_e = np.concatenate(
        [Wg.T.reshape(DT, 128, NMEL), np.ones((DT, 128, 1), dtype=f32)],
        axis=2,
    )
    wgt_t = _bf16(wgt_e.transpose(1, 0, 2))
    c1 = (Wmel @ ln_g).astype(f32)
    c2 = (Wmel @ ln_b + bmel).astype(f32)

    cb_base = np.zeros((128, CB_LEN), dtype=f32)
    b1_cols = np.asarray(inputs["conv1_b"], dtype=f32).reshape(DT, 128).T
    b2_cols = np.asarray(inputs["conv2_b"], dtype=f32).reshape(DT, 128).T
    cb_base[:, CB_B1 : CB_B1 + DT] = b1_cols
    cb_base[:, CB_B2S : CB_B2S + DT] = b2_cols * np.float32(0.03125)
    cb_base[:, CB_B2 : CB_B2 + DT] = b2_cols
    cb_base[:, CB_EPS] = LN_EPS
    cb_base[:, CB_BBIG : CB_BBIG + DT] = b_big64.astype(f32).reshape(DT, 128).T
    cb_base[0:NMEL, CB_C2] = c2
    cb_base[:, CB_WSC : CB_WSC + DT] = wscale.T

    llama = np.asarray(inputs["llama_embeddings"], dtype=f32).reshape(B * T, L)
    conv1_w_np = np.asarray(inputs["conv1_w"], dtype=np.float64)  # [D, GS, 3]
    conv1_b_np = np.asarray(inputs["conv1_b"], dtype=np.float64)
    gidx = np.arange(D) // GS

    import math
    _erf_vec = np.vectorize(math.erf)

    def _gelu64(x):
        return x * 0.5 * (1.0 + _erf_vec(x / math.sqrt(2.0)))

    shared = dict(wbig=wbig_t, wbig8=wbig8_t, cw1=cw1_t, cw2=cw2_t,
                  wgt=wgt_t, onec=_bf16(np.ones((128, 1), dtype=f32)))
    per_core = []
    for c in range(NCORES):
        b, h = divmod(c, 2)
        start = b * T + h * TOK
        ext_idx = np.arange(start - 2, start + TOK + 2)
        valid = (ext_idx >= b * T) & (ext_idx < (b + 1) * T)
        xext = np.zeros((EXT, L), dtype=f32)
        xext[valid] = llama[ext_idx[valid]]
        xt = _bf16(
            xext.T.reshape(2, KH, 128, EXT).transpose(0, 2, 1, 3)
        )  # [j, p, kk, t]
        xt8 = np.asarray(xt[0][:, 0:4, :], dtype=np.float32).astype(
            ml_dtypes.float8_e4m3
        )

        # host-computed halo columns (exact fp32-grade)
        def fcol(u):
            gu = start + u
            if b * T <= gu < (b + 1) * T:
                return llama[gu].astype(np.float64) @ W_big64 + b_big64
            return np.zeros(D, dtype=np.float64)

        def conv1col(m3):
            # m3: [D, 3] inputs for taps 0..2 -> conv1 + bias, gelu
            in_g = m3.reshape(GROUPS_, GS, 3)[gidx]       # [D, GS, 3]
            out = np.einsum("cit,cit->c", conv1_w_np, in_g) + conv1_b_np
            return _gelu64(out)

        fm2, fm1, f0 = fcol(-2), fcol(-1), fcol(0)
        f510, f511 = fcol(510), fcol(511)
        f512, f513 = fcol(TOK), fcol(TOK + 1)
        if h == 1:
            g_left = conv1col(np.stack([fm2, fm1, f0], axis=1))
        else:
            g_left = np.zeros(D, dtype=np.float64)
        if h == 0:
            g_right = conv1col(np.stack([f511, f512, f513], axis=1))
        else:
            g_right = np.zeros(D, dtype=np.float64)
        halo = np.zeros((128, DT, 6), dtype=f32)
        for dd in range(DT):
            slc = slice(dd * 128, (dd + 1) * 128)
            halo[:, dd, 0] = fm2[slc]
            halo[:, dd, 1] = fm1[slc]
            halo[:, dd, 2] = f512[slc]
            halo[:, dd, 3] = f513[slc]
            halo[:, dd, 4] = g_left[slc]
            halo[:, dd, 5] = g_right[slc]

        smb = np.zeros((1, 2 * NMEL), dtype=f32)
        smb[0, 0:NMEL] = c1
        smb[0, NMEL : 2 * NMEL] = 1.0

        per_core.append(dict(xt=xt, xt8=xt8, cb=cb_base, halo=_bf16(halo),
                             smb=_bf16(smb)))
    return shared, per_core


def _ensure_axon_hooks():
    """If this image's antenv lacks axon_hooks (needed by bass_utils when
    BASS_TRACE is set under axon), register a functional ctypes-based hook so
    tracing degrades gracefully instead of crashing."""
    try:
        import antenv.axon_hooks  # noqa: F401
        return
    except ImportError:
        pass
    try:
        import contextlib
        import ctypes
        import types

        hook = None
        try:
            lib = ctypes.CDLL("/opt/axon/libaxon_pjrt.so")
            if hasattr(lib, "axon_start_nrt_profile"):
                lib.axon_start_nrt_profile.argtypes = [
                    ctypes.POINTER(ctypes.c_int64),
                    ctypes.c_size_t,
                ]
                lib.axon_start_nrt_profile.restype = ctypes.c_int64
                lib.axon_stop_nrt_profile.argtypes = [ctypes.c_char_p]
                lib.axon_stop_nrt_profile.restype = ctypes.c_int64

                @contextlib.contextmanager
                def hook(output_dir, device_ids):
                    import jax

                    jax.devices()
                    if device_ids:
                        ids = (ctypes.c_int64 * len(device_ids))(*device_ids)
                        rc = lib.axon_start_nrt_profile(ids, len(device_ids))
                    else:
                        rc = lib.axon_start_nrt_profile(None, 0)
                    if rc != 0:
                        raise RuntimeError(f"axon_start_nrt_profile rc={rc}")
                    try:
                        yield
                    finally:
                        lib.axon_stop_nrt_profile(str(output_dir).encode())
        except OSError:
            hook = None

        mod = types.ModuleType("antenv.axon_hooks")
        mod.get_axon_ntff_profile_hook = lambda: hook
        mod.set_axon_ntff_profile_hook = lambda h: None
        sys.modules["antenv.axon_hooks"] = mod
        import antenv

        antenv.axon_hooks = mod
    except Exception:
        pass


def kernel(**inputs):
    global _PROGRAM, LAST_RESULTS
    _ensure_concourse()
    _ensure_axon_hooks()
    from concourse import bass_utils

    if _PROGRAM is None:
        _PROGRAM = _build_program()
    nc = _PROGRAM

    shared, per_core = host_prep(inputs)
    in_maps = [{**shared, **pc} for pc in per_core]

    res = None
    last_exc = None
    for _attempt in range(3):
        try:
            res = bass_utils.run_bass_kernel_spmd(
                nc, in_maps, core_ids=list(range(NCORES))
            )
            break
        except Exception as exc:  # transient NRT device errors happen
            last_exc = exc
    if res is None:
        raise last_exc
    LAST_RESULTS = res

    out = np.zeros((B, NMEL, T), dtype=np.float32)
    for c in range(NCORES):
        b, h = divmod(c, 2)
        out[b, :, h * TOK : (h + 1) * TOK] = np.asarray(
            res.results[c]["mel"], dtype=np.float32)
    return out


# revision 83
# speedup vs baseline: 1.1643x; 1.1643x over previous
"""Trainium2 Bass kernel for the CSMAdapter module.

Contract: kernel(**inputs) takes the FULL unsharded inputs (as produced by
the reference setup_inputs()) and returns the FULL output [4, 100, 1024].

Strategy
--------
All weight-only computation is folded on the host (it is data-independent):
    w_proj   = W_in @ Wd.T + bd
    w_prime  = P.T @ w_proj @ P
    masked_w = w_prime * sigmoid(spectral_mask)
    A        = P @ masked_w.T @ P.T          # fused = x @ A
    W_big    = W_in.T @ A                    # fused = llama @ W_big + b_in @ A
The final LayerNorm + mel projection algebra is folded into the mel GEMM:
    mel[m,t] = rstd[t]*((Wg @ h2)[m,t] - mu[t]*c1[m]) + c2[m]
with Wg = Wmel * ln_g, c1 = Wmel @ ln_g, c2 = Wmel @ ln_b + bmel.
The -mu*c1 term is accumulated into the mel PSUM as a rank-1 matmul
(c1 (x) -mu/D*Dsum) so the tail is: rstd broadcast -> one multiply ->
one per-partition bias add -> output DMA.

Device (SPMD over 8 cores, data-parallel over the 4096 tokens, 512 each +
2-token conv halos).  All heavy matmuls run in bf16; PSUM stays fp32.

W_big chunks for d0/d1, d2/d3 and d6/d7 are stored fp8 (e4m3) with
per-output-channel scales folded into the fu dequant (the PE multiplies
fp8 stationary x bf16 moving at full rate), halving most of the weight
DMA; total quantization error is ~1.5e-2 of scale vs the 2e-2 gate.

DMA: the two HW DGE queues (sync + scalar) stream wbc0 k-sliced and the x
sub-chunks interleaved so the first GEMMs start right as the warmup ends;
then the rest of x, wbc1, wbc2.  The gpsimd software-DGE queue carries
wbc3 (d6/d7 fill the x-paced head), consts (halo columns land directly in
the fuall/gall slabs), then conv/mel weights.  All queues share the 16
physical DMA engines (~210 GB/s aggregate), so ordering (not lane count)
is what matters.  The LN tail computes rstd = sqrt(recip(var+eps)) with
recip on the vector engine back-to-back after var, and the Sqrt act table
is pre-warmed by a dummy activation whose input aliases the last Gelu's
output (so the dependency scheduler cannot hoist it before the Gelu-table
load, which would re-evict Sqrt).
"""

import sys

import numpy as np


def _ensure_concourse():
    try:
        import concourse  # noqa: F401
    except ImportError:  # pragma: no cover
        for p in ("/opt/trn_rl_repo", "/root/.axon_site/_ro/trn_rl_repo"):
            if p not in sys.path:
                sys.path.insert(0, p)


# ---- static shapes ----
B, T, L, D = 4, 1024, 3072, 1024
NCORES = 8
TOK = 512            # owned tokens per core
EXT = TOK + 4        # fused ext window: tokens -2 .. TOK+2
G1E = TOK + 2        # conv1 ext output: tokens -1 .. TOK+1
KT = L // 128        # 24
KH = KT // 2         # 12
DT = D // 128        # 8
NMEL = 100
GS = 64              # group size (1024 / 16 groups)
GROUPS_ = 16

# cb (per-partition constants) column layout
CB_B1 = 0            # conv1 bias (gelu bias), 8 cols
CB_B2S = 8           # conv2 bias / 32 (Square path), 8 cols
CB_B2 = 16           # conv2 bias, 8 cols
CB_EPS = 24          # LN eps
CB_BBIG = 25         # fused GEMM bias b_big, 8 cols
CB_C2 = 33           # mel output bias c2 (partitions 0..99)
CB_WSC = 34          # fp8 weight dequant scale per d-tile, 8 cols
CB_LEN = 42

# d-tiles whose W_big columns are stored fp8 (e4m3, per-column scaled).
# Three of four chunks keeps the added quantization error at ~1.5e-2 of
# scale (vs the 2e-2 gate, measured 1.24e-2 with two chunks); these are the
# front/mid-of-stream tiles so the DMA-bound kernel head shrinks the most.
FP8_CHUNKS = (0, 1, 3)       # wbig chunks for d0/d1, d2/d3, d6/d7

LN_EPS = 1e-5
N_WU = 0             # PE warmup matmuls (pstate ramp during input DMA)

_PROGRAM = None          # cached program
LAST_RESULTS = None      # BassKernelResults of the most recent run (for test.py)


def _build_program():
    _ensure_concourse()
    from concourse import bacc, tile
    import concourse.mybir as mybir

    f32 = mybir.dt.float32
    f32r = mybir.dt.float32r
    bf16 = mybir.dt.bfloat16
    fp8 = mybir.dt.float8e4
    AF = mybir.ActivationFunctionType
    MUL = mybir.AluOpType.mult
    SUB = mybir.AluOpType.subtract

    nc = bacc.Bacc("TRN2", debug=False, target_bir_lowering=False)

    # DRAM layouts are partition-major so every DMA is contiguous.
    # wbig chunk layout is k-major: [chunk, kp, ktile, dpair, dcol]
    xt_d = nc.dram_tensor("xt", [2, 128, KH, EXT], bf16, kind="ExternalInput")
    xt8_d = nc.dram_tensor("xt8", [128, 4, EXT], fp8, kind="ExternalInput")
    wbig8_d = nc.dram_tensor("wbig8", [3, 128, KT, 2, 128], fp8,
                             kind="ExternalInput")
    wbig_d = nc.dram_tensor("wbig", [1, 128, KT, 2, 128], bf16,
                            kind="ExternalInput")
    cw1_d = nc.dram_tensor("cw1", [128, DT, 3, 128], bf16, kind="ExternalInput")
    cw2_d = nc.dram_tensor("cw2", [128, DT, 3, 128], bf16, kind="ExternalInput")
    wgt_d = nc.dram_tensor("wgt", [128, DT, NMEL + 1], bf16,
                           kind="ExternalInput")
    cb_d = nc.dram_tensor("cb", [128, CB_LEN], f32, kind="ExternalInput")
    smb_d = nc.dram_tensor("smb", [1, 2 * NMEL], bf16, kind="ExternalInput")
    onec_d = nc.dram_tensor("onec", [128, 1], bf16, kind="ExternalInput")
    # host-computed halo columns: per d-tile, 4 fused halo cols + 2 g halo cols
    halo_d = nc.dram_tensor("halo", [128, DT, 6], bf16, kind="ExternalInput")
    mel_d = nc.dram_tensor("mel", [NMEL, TOK], bf16, kind="ExternalOutput")

    with tile.TileContext(nc) as tc:
        with (
            tc.tile_pool(name="consts", bufs=1) as consts,
            tc.tile_pool(name="wpool", bufs=1) as wpool,
            tc.tile_pool(name="acts", bufs=1) as acts,
            tc.tile_pool(name="stats", bufs=1) as stats,
            tc.tile_pool(name="ps_mm", bufs=6, space="PSUM") as ps_mm,
            tc.tile_pool(name="ps_sq", bufs=1, space="PSUM") as ps_sqp,
            tc.tile_pool(name="ps_mel", bufs=1, space="PSUM") as ps_melp,
        ):
            fuall = acts.tile([128, DT, EXT], bf16, name="fuall")
            gall = acts.tile([128, DT, G1E], bf16, name="gall")

            # ---- input DMAs (see module docstring) ----
            def load_split(t, src_ap):
                nc.sync.dma_start(out=t[0:64], in_=src_ap[0:64])
                nc.scalar.dma_start(out=t[64:128], in_=src_ap[64:128])

            wbc = [
                wpool.tile([128, KT, 2, 128],
                           fp8 if c in FP8_CHUNKS else bf16,
                           name=f"wbc{c}", tag=f"wbc{c}")
                for c in range(4)
            ]
            xg = [
                consts.tile([128, KH, EXT], bf16, name=f"xg{j}", tag=f"xg{j}")
                for j in range(2)
            ]
            cw1_sb = consts.tile([128, DT, 3, 128], bf16, name="cw1_sb")
            cw2_sb = consts.tile([128, DT, 3, 128], bf16, name="cw2_sb")
            wgt_sb = consts.tile([128, DT, NMEL + 1], bf16, name="wgt_sb")

            # HW queues: wbc0 (fp8) k-sliced against the first x sub-chunks
            # so the d0/d1 GEMMs can start as soon as the warmup ends, then
            # the rest of x, then wbc1 (fp8) and wbc2 (bf16).
            xg8 = consts.tile([128, 4, EXT], fp8, name="xg8")
            load_split(xg8, xt8_d[:])
            load_split(wbc[0][:, 0:8], wbig8_d[0][:, 0:8])
            # bf16 copy of the fp8 k0-3 chunk for the later (bf16-lhsT)
            # d-tiles; runs on the idle vector engine right after arrival.
            nc.vector.tensor_copy(xg[0][:, 0:4, :], xg8)
            load_split(xg[0][:, 4:8, :], xt_d[0][:, 4:8, :])
            load_split(wbc[0][:, 8:16], wbig8_d[0][:, 8:16])
            load_split(xg[0][:, 8:12, :], xt_d[0][:, 8:12, :])
            load_split(xg[1][:, 0:4, :], xt_d[1][:, 0:4, :])
            load_split(wbc[0][:, 16:24], wbig8_d[0][:, 16:24])
            load_split(xg[1][:, 4:8, :], xt_d[1][:, 4:8, :])
            load_split(xg[1][:, 8:12, :], xt_d[1][:, 8:12, :])
            load_split(wbc[1], wbig8_d[1])
            load_split(wbc[2], wbig_d[0])
            # gpsimd lane: wbc3 (fp8) first so d6/d7 can fill the x-paced
            # head, then the consts (needed from fu0 ~28us), then conv/mel
            # weights.
            nc.gpsimd.dma_start(out=wbc[3], in_=wbig8_d[2])
            cb_sb = consts.tile([128, CB_LEN], f32, name="cb_sb")
            nc.gpsimd.dma_start(out=cb_sb, in_=cb_d[:])
            ones_col = consts.tile([128, 1], bf16, name="ones_col")
            nc.gpsimd.dma_start(out=ones_col, in_=onec_d[:])
            smb_sb = consts.tile([1, 2 * NMEL], bf16, name="smb_sb")
            nc.gpsimd.dma_start(out=smb_sb, in_=smb_d[:])
            # host halo columns DMAed straight into the fused / conv1 slabs
            # (replaces 32 small vector copies in the conv phase).
            nc.gpsimd.dma_start(out=fuall[:, :, 0:2], in_=halo_d[:, :, 0:2])
            nc.gpsimd.dma_start(out=fuall[:, :, EXT - 2 : EXT],
                                in_=halo_d[:, :, 2:4])
            nc.gpsimd.dma_start(out=gall[:, :, 0:1], in_=halo_d[:, :, 4:5])
            nc.gpsimd.dma_start(out=gall[:, :, G1E - 1 : G1E],
                                in_=halo_d[:, :, 5:6])
            nc.gpsimd.dma_start(out=cw1_sb, in_=cw1_d[:])
            nc.gpsimd.dma_start(out=cw2_sb, in_=cw2_d[:])
            nc.gpsimd.dma_start(out=wgt_sb, in_=wgt_d[:])

            def xk(k):
                return xg[k // KH][:, k % KH, :]

            h2 = [None] * DT
            h2sq = [None] * DT
            ps_sq_ref = [None]
            ps_m_ref = [None]
            psA = {}

            def gemm_chunk(d, ka, kb):
                if d not in psA:
                    psA[d] = ps_mm.tile([128, TOK], f32, name=f"psA{d}",
                                        tag="mm")
                for k in range(ka, kb):
                    if k < 4 and (d // 2) in (0, 3):
                        # fp8 lhsT tiles consume the fp8 first-chunk rhs
                        # directly (no wait on the bf16 cast)
                        rhs = xg8[:, k, 2 : 2 + TOK]
                    else:
                        rhs = xk(k)[:, 2 : 2 + TOK]
                    nc.tensor.matmul(
                        psA[d],
                        lhsT=wbc[d // 2][:, k, d % 2, :],
                        rhs=rhs,
                        start=(k == 0), stop=(k == KT - 1),
                    )

            def fu_copy(d, eng=None):
                # fu = psA * wscale + b_big; wscale is the fp8 per-channel
                # dequant scale (1.0 for the bf16 d-tiles).  Late tiles run
                # on the vector engine to debottleneck the scalar engine in
                # the conv phase.
                if eng is None:
                    nc.scalar.activation(
                        out=fuall[:, d, 2 : 2 + TOK], in_=psA[d],
                        func=AF.Identity,
                        bias=cb_sb[:, CB_BBIG + d : CB_BBIG + d + 1],
                        scale=cb_sb[:, CB_WSC + d : CB_WSC + d + 1],
                    )
                else:
                    eng.tensor_scalar(
                        fuall[:, d, 2 : 2 + TOK], psA[d],
                        cb_sb[:, CB_WSC + d : CB_WSC + d + 1],
                        cb_sb[:, CB_BBIG + d : CB_BBIG + d + 1],
                        MUL, mybir.AluOpType.add,
                    )

            def conv1(d):
                # device computes g_ext cols [1, 513); cols 0 and 513 from host
                ps = ps_mm.tile([128, TOK], f32, name=f"psB{d}", tag="mm")
                for tap in range(3):
                    nc.tensor.matmul(
                        ps, lhsT=cw1_sb[:, d, tap, :],
                        rhs=fuall[:, d, 1 + tap : 1 + tap + TOK],
                        start=(tap == 0), stop=(tap == 2),
                    )
                nc.scalar.activation(
                    out=gall[:, d, 1 : 1 + TOK], in_=ps, func=AF.Gelu,
                    bias=cb_sb[:, CB_B1 + d : CB_B1 + d + 1], scale=1.0,
                )

            def conv2(d):
                h2d = acts.tile([128, TOK], bf16, name=f"h2{d}", tag=f"h2{d}")
                h2sqd = acts.tile([128, TOK], bf16, name=f"h2sq{d}", tag="h2sq",
                                  bufs=2)
                h2[d] = h2d
                h2sq[d] = h2sqd
                ps = ps_mm.tile([128, TOK], f32, name=f"psC{d}", tag="mm")
                for tap in range(3):
                    nc.tensor.matmul(
                        ps, lhsT=cw2_sb[:, d, tap, :],
                        rhs=gall[:, d, tap : tap + TOK],
                        start=(tap == 0), stop=(tap == 2),
                    )
                nc.vector.tensor_scalar_add(
                    h2d, ps, cb_sb[:, CB_B2 + d : CB_B2 + d + 1])
                # h2sq = ((ps + b2)/32)^2 = h2^2 / 1024 -> ps_sq row = E[h2^2]
                nc.scalar.activation(
                    out=h2sqd, in_=ps, func=AF.Square,
                    bias=cb_sb[:, CB_B2S + d : CB_B2S + d + 1], scale=0.03125,
                )

            stat_n = [0]

            def statmm(d):
                # token-wise sums of h2 and h2^2; the mel matmuls are
                # deferred past the stats so the LN chain starts earlier.
                first = stat_n[0] == 0
                stat_n[0] += 1
                last = stat_n[0] == DT
                if first:
                    ps_sq_ref[0] = ps_sqp.tile([33, TOK], f32, name="ps_sq")
                nc.tensor.matmul(ps_sq_ref[0][0:1, :], lhsT=ones_col,
                                 rhs=h2sq[d][:], start=first, stop=last)
                nc.tensor.matmul(ps_sq_ref[0][32:33, :], lhsT=ones_col,
                                 rhs=h2[d][:], start=first, stop=last)

            # ---- emission in expected DMA-arrival order ----
            # d0/d1 interleave against the early x sub-chunks; d6/d7 (whose
            # weights arrive early on the gpsimd lane) join to fill the gaps
            # between x sub-chunk arrivals; d2..d5 follow weight arrival.
            gemm_chunk(0, 0, 4)
            gemm_chunk(1, 0, 4)
            gemm_chunk(0, 4, 8)
            gemm_chunk(1, 4, 8)
            gemm_chunk(6, 0, 4)
            gemm_chunk(7, 0, 4)
            gemm_chunk(6, 4, 8)
            gemm_chunk(7, 4, 8)
            gemm_chunk(0, 8, 12)
            gemm_chunk(1, 8, 12)
            gemm_chunk(6, 8, 12)
            gemm_chunk(7, 8, 12)
            for c in (3, 4, 5):
                for d in (0, 1, 6, 7):
                    gemm_chunk(d, 4 * c, 4 * c + 4)
            fu_copy(0)
            fu_copy(1)
            fu_copy(6)
            fu_copy(7)
            conv1(0)
            gemm_chunk(2, 0, KT)
            fu_copy(2, eng=nc.vector)
            conv1(1)
            gemm_chunk(3, 0, KT)
            fu_copy(3, eng=nc.vector)
            conv1(6)
            conv2(0)
            gemm_chunk(4, 0, KT)
            fu_copy(4, eng=nc.vector)
            conv1(7)
            conv2(1)
            statmm(0)
            gemm_chunk(5, 0, KT)
            fu_copy(5, eng=nc.vector)
            conv1(2)
            conv2(6)
            statmm(1)
            conv1(3)
            conv2(7)
            statmm(6)
            conv1(4)
            conv2(2)
            statmm(7)
            conv1(5)
            # pre-warm the Sqrt act table right after the last Gelu - the
            # input is gall's d5 slice so the dependency-driven scheduler
            # cannot hoist this above conv1(5)'s Gelu (whose own table load
            # would re-evict Sqrt).  Square is present in every table set so
            # the remaining h2sq squares are unaffected, and the LN-tail
            # Sqrt then needs no load on the critical path.
            pre = stats.tile([1, 8], f32, name="pre", tag="pre", bufs=2)
            nc.scalar.activation(pre, gall[0:1, 5, 0:8], AF.Sqrt,
                                 bias=cb_sb[0:1, CB_EPS : CB_EPS + 1],
                                 scale=1.0)
            conv2(3)
            statmm(2)
            conv2(4)
            statmm(3)
            conv2(5)
            statmm(4)
            statmm(5)

            # ---- deferred mel matmuls (overlap the LN stats chain) ----
            ps_m = ps_melp.tile([NMEL, TOK], f32, name="ps_m")
            ps_m_ref[0] = ps_m
            for i, d in enumerate(range(DT)):
                nc.tensor.matmul(ps_m, lhsT=wgt_sb[:, d, 0:NMEL],
                                 rhs=h2[d][:], start=(i == 0), stop=False)

            # ---- LN stats on [1, TOK] ----
            ps_sq = ps_sq_ref[0][0:1, :]     # E[h2^2] per token
            mu_row = ps_sq_ref[0][32:33, :]  # sum h2; x(1/D) folded into ops
            negmu = stats.tile([1, TOK], bf16, name="negmu")
            msq = stats.tile([1, TOK], f32, name="msq")
            var = stats.tile([1, TOK], f32, name="var", tag="sv", bufs=2)
            rvar = stats.tile([1, TOK], f32, name="rvar", tag="sv", bufs=2)
            rstd = stats.tile([1, TOK], bf16, name="rstd")
            # column-halved software pipeline; var and recip are back-to-back
            # on the vector engine, and the final Sqrt writes bf16 directly:
            #   rstd = sqrt(1/(E[h^2] + eps - mu^2))
            HT = TOK // 2
            Q3 = HT + TOK // 4
            for h in range(2):
                s = slice(h * HT, (h + 1) * HT)
                nc.scalar.activation(msq[0:1, s], mu_row[0:1, s], AF.Square,
                                     scale=1.0 / D)
                nc.vector.scalar_tensor_tensor(
                    var[0:1, s], in0=ps_sq[0:1, s], scalar=LN_EPS,
                    in1=msq[0:1, s], op0=mybir.AluOpType.add, op1=SUB,
                )
                nc.vector.reciprocal_approx_fast(rvar[0:1, s], var[0:1, s])
                nc.scalar.activation(rstd[0:1, s], rvar[0:1, s], AF.Sqrt,
                                     bias=0.0, scale=1.0)

            # negmu = -mu in bf16 feeds the rank-1 c1 correction; emitted
            # after the rstd chain so the in-order vector queue does not
            # delay var h0.
            nc.vector.tensor_scalar_mul(negmu, mu_row, -1.0 / D)
            nc.tensor.matmul(
                ps_m[0:NMEL, :], lhsT=smb_sb[0:1, 0:NMEL],
                rhs=negmu, start=False, stop=True,
            )

            # ---- rstd broadcast + output assembly (2-half pipeline) ----
            ps_s = ps_mm.tile([NMEL, TOK], f32, name="ps_s", tag="mm")
            s_sb = stats.tile([NMEL, TOK], bf16, name="s_sb")
            out_sb = stats.tile([NMEL, TOK], bf16, name="out_sb")
            c2col = cb_sb[0:NMEL, CB_C2 : CB_C2 + 1]
            for h in range(2):
                s = slice(h * HT, (h + 1) * HT)
                nc.tensor.matmul(
                    ps_s[:, s], lhsT=smb_sb[0:1, NMEL : 2 * NMEL],
                    rhs=rstd[0:1, s], start=True, stop=True,
                )
                nc.scalar.copy(s_sb[:, s], ps_s[:, s])
                nc.vector.tensor_mul(out_sb[:, s], ps_m[0:NMEL, s],
                                     s_sb[:, s])
                nc.scalar.add(out=out_sb[:, s], in_=out_sb[:, s], add=c2col)
            nc.sync.dma_start(out=mel_d[:, 0:HT], in_=out_sb[:, 0:HT])
            nc.scalar.dma_start(out=mel_d[:, HT:Q3], in_=out_sb[:, HT:Q3])
            nc.gpsimd.dma_start(out=mel_d[:, Q3:TOK], in_=out_sb[:, Q3:TOK])

    nc.compile()
    return nc


def _sigmoid64(x):
    return 1.0 / (1.0 + np.exp(-x.astype(np.float64)))


def _bf16(a):
    import ml_dtypes

    return np.ascontiguousarray(np.asarray(a, dtype=np.float32)).astype(
        ml_dtypes.bfloat16
    )


def host_prep(inputs):
    """Fold all data-independent computation; build per-core device inputs.

    Returns (shared, per_core) where shared is a dict of replicated arrays
    and per_core is a list of 8 dicts with the core-specific arrays.
    """
    f32 = np.float32
    W_in = np.asarray(inputs["W_in"], dtype=np.float64)
    Wd = np.asarray(inputs["Wd"], dtype=np.float64)
    bd = np.asarray(inputs["bd"], dtype=np.float64)
    P = np.asarray(inputs["P"], dtype=np.float64)
    smask = np.asarray(inputs["spectral_mask"], dtype=np.float64)
    b_in = np.asarray(inputs["b_in"], dtype=np.float64)

    w_proj = W_in @ Wd.T + bd[None, :]
    w_prime = P.T @ w_proj @ P
    masked_w = w_prime * _sigmoid64(smask)
    A = P @ masked_w.T @ P.T
    W_big64 = W_in.T @ A                                       # [L, D] f64
    b_big64 = b_in @ A                                         # [D] f64
    W_big = np.ascontiguousarray(W_big64, dtype=f32)

    # [chunk of 2 d-tiles, kp, ktile, d%2, dc] (partition-major, k-major)
    import ml_dtypes

    wchunks = W_big.reshape(KT, 128, 4, 2, 128).transpose(2, 1, 0, 3, 4)
    # fp8 chunks: per-output-column scales (folded into the fu dequant)
    wscale = np.ones((DT, 128), dtype=f32)
    w8_list = []
    w16_list = []
    for c in range(4):
        if c in FP8_CHUNKS:
            cols64 = W_big64[:, c * 256 : (c + 1) * 256]     # [L, 256]
            amax = np.abs(cols64).max(axis=0)
            s = (amax / 224.0).astype(f32)
            s[s == 0] = 1.0
            wscale[2 * c] = s[0:128]
            wscale[2 * c + 1] = s[128:256]
            sc = s.reshape(2, 128)[None, None, :, :]          # [1,1,2,128]
            w8_list.append(
                (wchunks[c] / sc).astype(np.float32).astype(
                    ml_dtypes.float8_e4m3
                )
            )
        else:
            w16_list.append(_bf16(wchunks[c]))
    wbig8_t = np.stack(w8_list, axis=0)
    wbig_t = np.stack(w16_list, axis=0)

    def blockdiag(w):
        w = np.asarray(w, dtype=f32)  # [C, GS, 3]
        out = np.zeros((DT, 3, 128, 128), dtype=f32)
        for d in range(DT):
            for co in range(128):
                c = d * 128 + co
                blk = co // GS
                # out[d, tap, blk*GS + i, co] = w[c, i, tap]
                out[d, :, blk * GS : (blk + 1) * GS, co] = w[c].T
        return out

    cw1_t = _bf16(blockdiag(inputs["conv1_w"]).transpose(2, 0, 1, 3))
    cw2_t = _bf16(blockdiag(inputs["conv2_w"]).transpose(2, 0, 1, 3))

    Wmel = np.asarray(inputs["Wmel"], dtype=np.float64)
    ln_g = np.asarray(inputs["ln_g"], dtype=np.float64)
    ln_b = np.asarray(inputs["ln_b"], dtype=np.float64)
    bmel = np.asarray(inputs["bmel"], dtype=np.float64)
    Wg = (Wmel * ln_g[None, :]).astype(f32)                    # [NMEL, D]
    wgt_e = np.concatenate(
        [Wg.T.reshape(DT, 128, NMEL), np.ones((DT, 128, 1), dtype=f32)],
        axis=2,
    )
    wgt_t = _bf16(wgt_e.transpose(1, 0, 2))
    c1 = (Wmel @ ln_g).astype(f32)
    c2 = (Wmel @ ln_b + bmel).astype(f32)

    cb_base = np.zeros((128, CB_LEN), dtype=f32)
    b1_cols = np.asarray(inputs["conv1_b"], dtype=f32).reshape(DT, 128).T
    b2_cols = np.asarray(inputs["conv2_b"], dtype=f32).reshape(DT, 128).T
    cb_base[:, CB_B1 : CB_B1 + DT] = b1_cols
    cb_base[:, CB_B2S : CB_B2S + DT] = b2_cols * np.float32(0.03125)
    cb_base[:, CB_B2 : CB_B2 + DT] = b2_cols
    cb_base[:, CB_EPS] = LN_EPS
    cb_base[:, CB_BBIG : CB_BBIG + DT] = b_big64.astype(f32).reshape(DT, 128).T
    cb_base[0:NMEL, CB_C2] = c2
    cb_base[:, CB_WSC : CB_WSC + DT] = wscale.T

    llama = np.asarray(inputs["llama_embeddings"], dtype=f32).reshape(B * T, L)
    conv1_w_np = np.asarray(inputs["conv1_w"], dtype=np.float64)  # [D, GS, 3]
    conv1_b_np = np.asarray(inputs["conv1_b"], dtype=np.float64)
    gidx = np.arange(D) // GS

    import math
    _erf_vec = np.vectorize(math.erf)

    def _gelu64(x):
        return x * 0.5 * (1.0 + _erf_vec(x / math.sqrt(2.0)))

    shared = dict(wbig=wbig_t, wbig8=wbig8_t, cw1=cw1_t, cw2=cw2_t,
                  wgt=wgt_t, onec=_bf16(np.ones((128, 1), dtype=f32)))
    per_core = []
    for c in range(NCORES):
        b, h = divmod(c, 2)
        start = b * T + h * TOK
        ext_idx = np.arange(start - 2, start + TOK + 2)
        valid = (ext_idx >= b * T) & (ext_idx < (b + 1) * T)
        xext = np.zeros((EXT, L), dtype=f32)
        xext[valid] = llama[ext_idx[valid]]
        xt = _bf16(
            xext.T.reshape(2, KH, 128, EXT).transpose(0, 2, 1, 3)
        )  # [j, p, kk, t]
        xt8 = np.asarray(xt[0][:, 0:4, :], dtype=np.float32).astype(
            ml_dtypes.float8_e4m3
        )

        # host-computed halo columns (exact fp32-grade)
        def fcol(u):
            gu = start + u
            if b * T <= gu < (b + 1) * T:
                return llama[gu].astype(np.float64) @ W_big64 + b_big64
            return np.zeros(D, dtype=np.float64)

        def conv1col(m3):
            # m3: [D, 3] inputs for taps 0..2 -> conv1 + bias, gelu
            in_g = m3.reshape(GROUPS_, GS, 3)[gidx]       # [D, GS, 3]
            out = np.einsum("cit,cit->c", conv1_w_np, in_g) + conv1_b_np
            return _gelu64(out)

        fm2, fm1, f0 = fcol(-2), fcol(-1), fcol(0)
        f510, f511 = fcol(510), fcol(511)
        f512, f513 = fcol(TOK), fcol(TOK + 1)
        if h == 1:
            g_left = conv1col(np.stack([fm2, fm1, f0], axis=1))
        else:
            g_left = np.zeros(D, dtype=np.float64)
        if h == 0:
            g_right = conv1col(np.stack([f511, f512, f513], axis=1))
        else:
            g_right = np.zeros(D, dtype=np.float64)
        halo = np.zeros((128, DT, 6), dtype=f32)
        for dd in range(DT):
            slc = slice(dd * 128, (dd + 1) * 128)
            halo[:, dd, 0] = fm2[slc]
            halo[:, dd, 1] = fm1[slc]
            halo[:, dd, 2] = f512[slc]
            halo[:, dd, 3] = f513[slc]
            halo[:, dd, 4] = g_left[slc]
            halo[:, dd, 5] = g_right[slc]

        smb = np.zeros((1, 2 * NMEL), dtype=f32)
        smb[0, 0:NMEL] = c1
        smb[0, NMEL : 2 * NMEL] = 1.0

        per_core.append(dict(xt=xt, xt8=xt8, cb=cb_base, halo=_bf16(halo),
                             smb=_bf16(smb)))
    return shared, per_core


def _ensure_axon_hooks():
    """If this image's antenv lacks axon_hooks (needed by bass_utils when
    BASS_TRACE is set under axon), register a functional ctypes-based hook so
    tracing degrades gracefully instead of crashing."""
    try:
        import antenv.axon_hooks  # noqa: F401
        return
    except ImportError:
        pass
    try:
        import contextlib
        import ctypes
        import types

        hook = None
        try:
            lib = ctypes.CDLL("/opt/axon/libaxon_pjrt.so")
            if hasattr(lib, "axon_start_nrt_profile"):
                lib.axon_start_nrt_profile.argtypes = [
                    ctypes.POINTER(ctypes.c_int64),
                    ctypes.c_size_t,
                ]
                lib.axon_start_nrt_profile.restype = ctypes.c_int64
                lib.axon_stop_nrt_profile.argtypes = [ctypes.c_char_p]
                lib.axon_stop_nrt_profile.restype = ctypes.c_int64

                @contextlib.contextmanager
                def hook(output_dir, device_ids):
                    import jax

                    jax.devices()
                    if device_ids:
                        ids = (ctypes.c_int64 * len(device_ids))(*device_ids)
                        rc = lib.axon_start_nrt_profile(ids, len(device_ids))
                    else:
                        rc = lib.axon_start_nrt_profile(None, 0)
                    if rc != 0:
                        raise RuntimeError(f"axon_start_nrt_profile rc={rc}")
                    try:
                        yield
                    finally:
                        lib.axon_stop_nrt_profile(str(output_dir).encode())
        except OSError:
            hook = None

        mod = types.ModuleType("antenv.axon_hooks")
        mod.get_axon_ntff_profile_hook = lambda: hook
        mod.set_axon_ntff_profile_hook = lambda h: None
        sys.modules["antenv.axon_hooks"] = mod
        import antenv

        antenv.axon_hooks = mod
    except Exception:
        pass


def kernel(**inputs):
    global _PROGRAM, LAST_RESULTS
    _ensure_concourse()
    _ensure_axon_hooks()
    from concourse import bass_utils

    if _PROGRAM is None:
        _PROGRAM = _build_program()
    nc = _PROGRAM

    shared, per_core = host_prep(inputs)
    in_maps = [{**shared, **pc} for pc in per_core]

    res = None
    last_exc = None
    for _attempt in range(3):
        try:
            res = bass_utils.run_bass_kernel_spmd(
                nc, in_maps, core_ids=list(range(NCORES))
            )
            break
        except Exception as exc:  # transient NRT device errors happen
            last_exc = exc
    if res is None:
        raise last_exc
    LAST_RESULTS = res

    out = np.zeros((B, NMEL, T), dtype=np.float32)
    for c in range(NCORES):
        b, h = divmod(c, 2)
        out[b, :, h * TOK : (h + 1) * TOK] = np.asarray(
            res.results[c]["mel"], dtype=np.float32)
    return out
